# revision 35
# baseline (speedup 1.0000x reference)
"""GCN message-passing (GCNConv) on 8 Trainium2 NeuronCores via Bass/Tile.

Math (reference):
    deg[i] = |{e : row[e] == i}|
    h      = x @ W
    out[i] = sum_{e: row[e]==i} h[col[e]] / sqrt(deg[row[e]] * deg[col[e]])

The GCN norm is separable, so with r = rsqrt(deg):
    h2   = ((x * r[:, None]) @ W)                 (phase 1, on device; r folded
                                                   into x on the host)
    out  = r[:, None] * scatter_add(h2[col], row)  (phase 2)

Phase 2 is a pure gather + segment-sum (PE matmul against one-hot S matrices).

The per-edge gather runs on the GPSIMD SWDGE path whose descriptor generation
is effectively serial at ~2.6 ns/index aggregate -- it is the kernel's
critical path.  This version therefore:
  * pads index arrays with -1 and passes the real per-bin count through a
    register (the SWDGE contract: num_idxs_reg == #non-negative indices), so
    padding generates no DMA descriptors (~9% fewer);
  * runs phase 1 in bf16 (x pre-scaled by r and converted on the host), with
    evacuation split across the scalar and vector engines and DMA split
    across the sync (x reads) and scalar (h2 writes + consts) HW queues, so
    the gather tables are ready as early as possible;
  * splits the int16-limited gather table at ~50% so the A/B halves carry
    even traffic, and runs A_HEAD A-half gathers while the hi table is still
    being written;
  * pre-zeroes the gather buffers on the (otherwise idle) gpsimd engine:
    rows skipped by the -1 truncation stay finite, and S=0 masks them.

(A PREPARE_ONLY + trigger_dma variant that generates descriptors during
phase 1 was tried and reverted: prep-mode generation measured ~45% slower
per index than direct mode, and Tile's consumer waits land on DMASW lane
semaphores that prep-mode descriptors never increment, which needs post-
compile wait rewriting.  See the session notes.)
"""

import math

import numpy as np
import ml_dtypes

import concourse.bacc as bacc
import concourse.bass as bass
import concourse.mybir as mybir
import concourse.tile as tile
from concourse.bass_utils import run_bass_kernel_spmd
from concourse.tile import add_dep_helper

P = 128
NCORES = 8
FEAT = 128
EMBED = 128
F32 = mybir.dt.float32
F16 = mybir.dt.float16
BF16 = mybir.dt.bfloat16
I16 = mybir.dt.int16

A_HEAD = 5  # A-half gathers issued ahead of the B stream
GA_BUFS = A_HEAD + 3
GB_BUFS = 4


# ----------------------------------------------------------------------------
# host-side preprocessing (index plumbing + rsqrt(deg) scale factors)
# ----------------------------------------------------------------------------

def _assign_bins(deg, nbins):
    """Balanced assignment of output rows to bins (<=128 rows per bin).

    Round-based LPT: each round hands the highest-degree unassigned rows to
    the least-loaded bins that still have space.  Returns (bin_of, slot_of).
    """
    n = deg.shape[0]
    order = np.argsort(-deg, kind="stable")
    bin_of = np.empty(n, np.int32)
    slot_of = np.empty(n, np.int32)
    load = np.zeros(nbins, np.float64)
    count = np.zeros(nbins, np.int32)
    pos = 0
    while pos < n:
        avail = np.nonzero(count < P)[0]
        take = min(avail.size, n - pos)
        rows_rd = order[pos : pos + take]
        b = avail[np.argsort(load[avail], kind="stable")[:take]]
        bin_of[rows_rd] = b
        slot_of[rows_rd] = count[b]
        count[b] += 1
        load[b] += deg[rows_rd]
        pos += take
    return bin_of, slot_of


def _prepare(x, W, row, col, cpc, cw, sc):
    """Build all per-core device input arrays.  cpc = bins per core.

    The h2 table lives in DRAM in phase-1-chunk-major order: phase-1
    processes nodes in chunks of cw; within chunk k (ntile_k = cw/128 node
    tiles) node j = i*128+p (i = node tile, p = partition) sits at table row
    jp = k*cw + p*ntile_k + (i - k*cw/128) -- so each chunk's h2 write is one
    contiguous run per partition.  Gather indices are int16, so the table is
    split at jp = sc*cw; edges are routed by their half.  Index arrays are
    padded with -1: the SWDGE ucode drops trailing negative indices, so
    padding generates no DMA descriptors.
    """
    N = x.shape[0]
    E = row.shape[0]
    nbins = NCORES * cpc
    assert nbins * P >= N, (nbins, N)
    n_pad = math.ceil(N / P) * P
    nt = n_pad // P
    cwt = cw // P
    split_jp = min(sc * cw, n_pad)
    assert split_jp <= 2**15, (sc, cw)
    assert n_pad - split_jp <= 2**15, (sc, cw, n_pad)

    deg = np.bincount(row, minlength=N).astype(np.float64)
    assert deg.min() >= 1
    r = (1.0 / np.sqrt(deg)).astype(np.float32)

    bin_of, slot_of = _assign_bins(deg, nbins)

    # group edges by (destination bin, col table-half)
    eb = bin_of[row].astype(np.int64)
    ed = slot_of[row].astype(np.float32)
    ci = col // P
    ck = ci // cwt
    ntile_k = np.minimum(cwt, nt - ck * cwt)
    jp = ck * cw + (col % P) * ntile_k + (ci - ck * cwt)
    par = (jp >= split_jp).astype(np.int64)
    pidx = np.where(jp >= split_jp, jp - split_jp, jp).astype(np.int16)
    key = eb * 2 + par
    ordk = np.argsort(key, kind="stable")
    counts = np.bincount(key, minlength=nbins * 2)
    nba = math.ceil(int(counts[0::2].max()) / P)
    nbb = math.ceil(int(counts[1::2].max()) / P)
    assert nba > 0 and nbb > 0
    cap_a, cap_b = nba * P, nbb * P

    idx_a = np.full((nbins, cap_a), -1, np.int16)
    dlt_a = np.full((nbins, cap_a), -1.0, np.float32)
    idx_b = np.full((nbins, cap_b), -1, np.int16)
    dlt_b = np.full((nbins, cap_b), -1.0, np.float32)
    starts = np.concatenate([[0], np.cumsum(counts)])
    ks = key[ordk]
    pig = (np.arange(E) - starts[ks]).astype(np.int64)
    m = (ks & 1) == 0
    idx_a[ks[m] >> 1, pig[m]] = pidx[ordk][m]
    dlt_a[ks[m] >> 1, pig[m]] = ed[ordk][m]
    m = ~m
    idx_b[ks[m] >> 1, pig[m]] = pidx[ordk][m]
    dlt_b[ks[m] >> 1, pig[m]] = ed[ordk][m]
    # ucode truncates trailing -1 indices; keep >=1 real index per half so
    # the instruction never degenerates to zero descriptors
    idx_a[counts[0::2] == 0, 0] = 0
    idx_b[counts[1::2] == 0, 0] = 0

    nb = nba + nbb
    # deltas per bin: [P, nb] with delta[p, b] = slot of edge b*128+p (or -1)
    d_a = dlt_a.reshape(nbins, nba, P).transpose(0, 2, 1)
    d_b = dlt_b.reshape(nbins, nbb, P).transpose(0, 2, 1)
    dall = np.concatenate([d_a, d_b], axis=2)  # [nbins, P, nb]

    # gather indices per bin: wrapped in 16 partitions, replicated 8x
    def mk_idx(idx, cap):
        t = idx.reshape(nbins, cap // 16, 16).transpose(0, 2, 1)
        return np.tile(t, (1, 8, 1))  # [nbins, 128, cap // 16]

    idx_all = np.concatenate([mk_idx(idx_a, cap_a), mk_idx(idx_b, cap_b)], axis=2)

    # per-slot output scale
    rout_bins = np.zeros((nbins, P), np.float32)
    rout_bins[bin_of, slot_of] = r

    # real (non-padded) index count per bin half; the gather's count register
    # must match the number of non-negative indices exactly
    cnts = np.zeros((nbins, 2), np.int32)
    cnts[:, 0] = np.maximum(counts[0::2], 1)
    cnts[:, 1] = np.maximum(counts[1::2], 1)

    # phase-1 arrays: xT pre-scaled by rsqrt(deg), bf16
    xT = np.zeros((FEAT, n_pad), np.float32)
    xT[:, :N] = np.ascontiguousarray(x.T) * r[None, :]
    xT = xT.astype(ml_dtypes.bfloat16)
    iota = np.tile(np.arange(P, dtype=np.float16), (P, nb))

    idxc = nb * 8  # idx columns per chunk
    core_maps = []
    for dd in range(NCORES):
        b0, b1 = dd * cpc, (dd + 1) * cpc
        core_maps.append(
            {
                "xT": xT,
                "W": np.ascontiguousarray(W.astype(ml_dtypes.bfloat16)),
                "iota": iota,
                "delta": np.ascontiguousarray(
                    dall[b0:b1].transpose(1, 0, 2).reshape(P, cpc * nb)
                ).astype(np.float16),
                "idx": np.ascontiguousarray(
                    idx_all[b0:b1].transpose(1, 0, 2).reshape(P, cpc * idxc)
                ),
                "r_out": np.ascontiguousarray(rout_bins[b0:b1].T),
                "cnt": np.tile(cnts[b0:b1].reshape(1, cpc * 2), (P, 1)),
            }
        )

    gslot = bin_of.astype(np.int64) * P + slot_of.astype(np.int64)
    return core_maps, gslot, nba, nbb, n_pad


# ----------------------------------------------------------------------------
# device kernel
# ----------------------------------------------------------------------------

def _build(n_pad, cpc, nba, nbb, cw, sc):
    nt = n_pad // P
    split_jp = min(sc * cw, n_pad)
    nb = nba + nbb
    idxc = nb * 8

    nc = bacc.Bacc(None, target_bir_lowering=False, debug=False, num_swdge_queues=4)
    xT_d = nc.declare_dram_parameter("xT", [P, n_pad], BF16, isOutput=False)
    W_d = nc.declare_dram_parameter("W", [P, EMBED], BF16, isOutput=False)
    io_d = nc.declare_dram_parameter("iota", [P, nb * P], F16, isOutput=False)
    dl_d = nc.declare_dram_parameter("delta", [P, cpc * nb], F16, isOutput=False)
    ix_d = nc.declare_dram_parameter("idx", [P, cpc * idxc], I16, isOutput=False)
    ro_d = nc.declare_dram_parameter("r_out", [P, cpc], F32, isOutput=False)
    cnt_d = nc.declare_dram_parameter("cnt", [P, cpc * 2], mybir.dt.int32, isOutput=False)
    out_d = nc.declare_dram_parameter("out", [cpc * P, EMBED], F32, isOutput=True)
    # chunk-major h2 rows (see _prepare docstring)
    h2_d = nc.dram_tensor("h2buf", [P * nt, EMBED], F16)

    starts = list(range(0, n_pad, cw))
    hi_starts = [s for s in starts if s >= split_jp]
    lo_starts = [s for s in starts if s < split_jp]

    with tile.TileContext(nc) as tc:
        with (
            tc.tile_pool(name="const", bufs=1) as constp,
            tc.tile_pool(name="ph1", bufs=4) as ph1,
            tc.tile_pool(name="ps1", bufs=4, space="PSUM") as ps1,
            tc.tile_pool(name="gA", bufs=GA_BUFS) as gAp,
            tc.tile_pool(name="gB", bufs=GB_BUFS) as gBp,
            tc.tile_pool(name="sS", bufs=3) as sSp,
            tc.tile_pool(name="ps2", bufs=4, space="PSUM") as ps2,
            tc.tile_pool(name="outp", bufs=4) as outp,
        ):
            W_sb = constp.tile([P, EMBED], BF16)
            io_sb = constp.tile([P, nb, P], F16)
            dl_sb = constp.tile([P, cpc * nb], F16)
            ix_sb = constp.tile([P, cpc * idxc], I16)
            ro_sb = constp.tile([P, cpc], F32)
            cnt_sb = constp.tile([P, cpc * 2], mybir.dt.int32)
            # consts all on the scalar HW queue: the sync queue starts
            # streaming x chunks immediately (phase 1 gates the gathers)
            nc.scalar.dma_start(out=W_sb[:], in_=W_d[:])
            nc.scalar.dma_start(out=cnt_sb[:], in_=cnt_d[:])
            nc.scalar.dma_start(out=ix_sb[:], in_=ix_d[:])
            nc.scalar.dma_start(
                out=io_sb[:], in_=io_d[:].rearrange("p (a b) -> p a b", b=P)
            )
            nc.scalar.dma_start(out=dl_sb[:], in_=dl_d[:])
            nc.scalar.dma_start(out=ro_sb[:], in_=ro_d[:])

            # zero the gather buffers once: trailing -1 indices generate no
            # DMA traffic, so those rows keep stale SBUF bytes -- they are
            # masked by S=0 but must stay finite (NaN*0 poisons PSUM).
            # gpsimd is idle until the first gather, so the memsets are free.
            for _ in range(GA_BUFS):
                z = gAp.tile([P, nba, EMBED], F16, tag="gA", name="gaz")
                nc.gpsimd.memset(z[:], 0.0)
            for _ in range(GB_BUFS):
                z = gBp.tile([P, nbb, EMBED], F16, tag="gB", name="gbz")
                nc.gpsimd.memset(z[:], 0.0)

            # ---------------- phase 1: h2 = (x*r) @ W  (bf16) ---------------
            # lo chunks first: the A-head gathers only need the lo table.
            hi_writes, lo_writes = [], []
            for start in lo_starts + hi_starts:
                w = min(cw, n_pad - start)
                ntile = w // P
                xt = ph1.tile([P, cw], BF16, tag="xt")
                nc.sync.dma_start(out=xt[:, :w], in_=xT_d[:, start : start + w])
                stage = ph1.tile([P, cw], F16, tag="stage")
                for gi, g0 in enumerate(range(0, ntile, 4)):
                    gn = min(4, ntile - g0)
                    ps = ps1.tile([P, 4, P], F32)
                    for i in range(gn):
                        t = g0 + i
                        nc.tensor.matmul(
                            ps[:, i, :],
                            xt[:, t * P : (t + 1) * P],
                            W_sb[:],
                            start=True,
                            stop=True,
                        )
                    st_view = stage[:, g0 * P : (g0 + gn) * P].rearrange(
                        "p (a b) -> p a b", b=P
                    )
                    # alternate evacuation engine so neither gates phase 1
                    if gi % 2 == 0:
                        nc.scalar.copy(st_view, ps[:, :gn, :])
                    else:
                        nc.vector.tensor_copy(st_view, ps[:, :gn, :])
                # contiguous chunk-major write (sync queue, interleaved with
                # the x reads; consts stay on the scalar queue so they never
                # delay the h2 writes that gate the gathers)
                wi = nc.sync.dma_start(
                    out=h2_d[start : start + w, :].rearrange(
                        "(p l) f -> p (l f)", l=ntile
                    ),
                    in_=stage[:, :w],
                )
                (hi_writes if start >= split_jp else lo_writes).append(wi.ins)

            # ---------------- phase 2: gather + segment-sum -----------------
            # Direct-mode SWDGE gathers.  A-half gathers wait on the lo table
            # (written first), B-half on the hi table; the first A_HEAD A
            # gathers run while the hi half of phase 1 still streams.
            lo_ap = h2_d[0:split_jp, :]
            hi_ap = h2_d[split_jp : P * nt, :]
            bar_lo = nc.sync.nop(hint="h2_lo_ready")
            for wi in lo_writes:
                add_dep_helper(bar_lo.ins, wi, reason="lo gathers wait on h2 lo")
            bar_hi = nc.sync.nop(hint="h2_hi_ready")
            for wi in hi_writes:
                add_dep_helper(bar_hi.ins, wi, reason="hi gathers wait on h2 hi")

            tiles = {}

            cregs = {}

            def issue_a(c):
                ga = gAp.tile([P, nba, EMBED], F16, tag="gA", name="ga")
                rA = nc.gpsimd.alloc_register(f"cA{c}")
                rB = nc.gpsimd.alloc_register(f"cB{c}")
                nc.gpsimd.reg_load([rA, rB], cnt_sb[0:1, 2 * c : 2 * c + 2])
                cregs[c] = rB
                g1 = nc.gpsimd.dma_gather(
                    ga[:],
                    lo_ap,
                    ix_sb[:, c * idxc : c * idxc + nba * 8],
                    nba * P,
                    rA,
                    EMBED,
                    single_packet=False,
                    queue_num=(2 * c) % 4,
                )
                add_dep_helper(g1.ins, bar_lo.ins, reason="h2 lo ready")
                return ga

            def issue_b(c):
                gb = gBp.tile([P, nbb, EMBED], F16, tag="gB", name="gb")
                rB = cregs.pop(c)
                g2 = nc.gpsimd.dma_gather(
                    gb[:],
                    hi_ap,
                    ix_sb[:, c * idxc + nba * 8 : (c + 1) * idxc],
                    nbb * P,
                    rB,
                    EMBED,
                    single_packet=False,
                    queue_num=(2 * c + 1) % 4,
                )
                add_dep_helper(g2.ins, bar_hi.ins, reason="h2 hi ready")
                return gb

            def consume(j):
                ga, gb = tiles.pop(j)
                S = sSp.tile([P, nb, P], F16, tag="S", name="S")
                dsl = (
                    dl_sb[:, j * nb : (j + 1) * nb]
                    .unsqueeze(2)
                    .to_broadcast([P, nb, P])
                )
                nc.vector.tensor_tensor(
                    out=S[:], in0=dsl, in1=io_sb[:], op=mybir.AluOpType.is_equal
                )
                ps = ps2.tile([P, EMBED], F32, tag="ps2t", name="ps2t")
                for b in range(nb):
                    rhs = ga[:, b, :] if b < nba else gb[:, b - nba, :]
                    nc.tensor.matmul(
                        ps[:],
                        S[:, b, :],
                        rhs,
                        start=(b == 0),
                        stop=(b == nb - 1),
                    )
                ot = outp.tile([P, EMBED], F32)
                nc.any.tensor_scalar(
                    out=ot[:],
                    in0=ps[:],
                    scalar1=ro_sb[:, j : j + 1],
                    scalar2=None,
                    op0=mybir.AluOpType.mult,
                )
                nc.scalar.dma_start(out=out_d[j * P : (j + 1) * P, :], in_=ot[:])

            # A_HEAD A-gathers up front (they only need the lo table), then
            # alternate B/A so neither table half ever starves the engine.
            ga_tiles = {}
            for c in range(min(A_HEAD, cpc)):
                ga_tiles[c] = issue_a(c)
            for c in range(cpc):
                gb = issue_b(c)
                if c + A_HEAD < cpc:
                    ga_tiles[c + A_HEAD] = issue_a(c + A_HEAD)
                tiles[c] = (ga_tiles.pop(c), gb)
                consume(c)

    nc.compile()
    return nc


# ----------------------------------------------------------------------------
# entry point
# ----------------------------------------------------------------------------

def _run(x, W, row, col, cpc=None, cw=2048, sc=12, trace=False):
    x = np.asarray(x, np.float32)
    W = np.asarray(W, np.float32)
    row = np.asarray(row).astype(np.int64)
    col = np.asarray(col).astype(np.int64)
    N = x.shape[0]
    if cpc is None:
        cpc = math.ceil(N / (NCORES * P))
    cw = min(cw, math.ceil(N / P) * P)
    core_maps, gslot, nba, nbb, n_pad = _prepare(x, W, row, col, cpc, cw, sc)
    nc = _build(n_pad, cpc, nba, nbb, cw, sc)
    res = run_bass_kernel_spmd(
        nc, core_maps, list(range(NCORES)), trace=trace
    )
    big = np.concatenate([res.results[d]["out"] for d in range(NCORES)], axis=0)
    out = np.ascontiguousarray(big[gslot], dtype=np.float32)
    return out, res


def kernel(**inputs):
    out, _ = _run(inputs["x"], inputs["W"], inputs["row"], inputs["col"])
    return out


# revision 36
# speedup vs baseline: 1.1665x; 1.1665x over previous
"""GCN message-passing (GCNConv) on 8 Trainium2 NeuronCores via Bass/Tile.

Math (reference):
    deg[i] = |{e : row[e] == i}|
    h      = x @ W
    out[i] = sum_{e: row[e]==i} h[col[e]] / sqrt(deg[row[e]] * deg[col[e]])

The GCN norm is separable, so with r = rsqrt(deg):
    h2   = ((x * r[:, None]) @ W)                 (phase 1, on device; r folded
                                                   into x on the host)
    out  = r[:, None] * scatter_add(h2[col], row)  (phase 2)

Phase 2 is a pure gather + segment-sum (PE matmul against one-hot S matrices).

The per-edge gather runs on the GPSIMD SWDGE path whose descriptor generation
is effectively serial at ~2.6 ns/index aggregate -- it is the kernel's
critical path.  This version therefore:
  * pads index arrays with -1 and passes the real per-bin count through a
    register (the SWDGE contract: num_idxs_reg == #non-negative indices), so
    padding generates no DMA descriptors (~9% fewer);
  * runs phase 1 in bf16 (x pre-scaled by r and converted on the host), with
    evacuation split across the scalar and vector engines and DMA split
    across the sync (x reads) and scalar (h2 writes + consts) HW queues, so
    the gather tables are ready as early as possible;
  * splits the int16-limited gather table at ~50% so the A/B halves carry
    even traffic, and runs A_HEAD A-half gathers while the hi table is still
    being written;
  * pre-zeroes the gather buffers on the (otherwise idle) gpsimd engine:
    rows skipped by the -1 truncation stay finite, and S=0 masks them.

(A PREPARE_ONLY + trigger_dma variant that generates descriptors during
phase 1 was tried and reverted: prep-mode generation measured ~45% slower
per index than direct mode, and Tile's consumer waits land on DMASW lane
semaphores that prep-mode descriptors never increment, which needs post-
compile wait rewriting.  See the session notes.)
"""

import math

import numpy as np
import ml_dtypes

import concourse.bacc as bacc
import concourse.bass as bass
import concourse.mybir as mybir
import concourse.tile as tile
from concourse.bass_utils import run_bass_kernel_spmd
from concourse.tile import add_dep_helper

P = 128
NCORES = 8
FEAT = 128
EMBED = 128
F32 = mybir.dt.float32
F16 = mybir.dt.float16
BF16 = mybir.dt.bfloat16
I16 = mybir.dt.int16

A_HEAD = 5  # A-half gathers issued ahead of the B stream
GA_BUFS = A_HEAD + 3
GB_BUFS = 4


# ----------------------------------------------------------------------------
# host-side preprocessing (index plumbing + rsqrt(deg) scale factors)
# ----------------------------------------------------------------------------

def _assign_bins(deg, nbins):
    """Balanced assignment of output rows to bins (<=128 rows per bin).

    Round-based LPT: each round hands the highest-degree unassigned rows to
    the least-loaded bins that still have space.  Returns (bin_of, slot_of).
    """
    n = deg.shape[0]
    order = np.argsort(-deg, kind="stable")
    bin_of = np.empty(n, np.int32)
    slot_of = np.empty(n, np.int32)
    load = np.zeros(nbins, np.float64)
    count = np.zeros(nbins, np.int32)
    pos = 0
    while pos < n:
        avail = np.nonzero(count < P)[0]
        take = min(avail.size, n - pos)
        rows_rd = order[pos : pos + take]
        b = avail[np.argsort(load[avail], kind="stable")[:take]]
        bin_of[rows_rd] = b
        slot_of[rows_rd] = count[b]
        count[b] += 1
        load[b] += deg[rows_rd]
        pos += take
    return bin_of, slot_of


def _prepare(x, W, row, col, cpc, cw, sc):
    """Build all per-core device input arrays.  cpc = bins per core.

    The h2 table lives in DRAM in phase-1-chunk-major order: phase-1
    processes nodes in chunks of cw; within chunk k (ntile_k = cw/128 node
    tiles) node j = i*128+p (i = node tile, p = partition) sits at table row
    jp = k*cw + p*ntile_k + (i - k*cw/128) -- so each chunk's h2 write is one
    contiguous run per partition.  Gather indices are int16, so the table is
    split at jp = sc*cw; edges are routed by their half.  Index arrays are
    padded with -1: the SWDGE ucode drops trailing negative indices, so
    padding generates no DMA descriptors.
    """
    N = x.shape[0]
    E = row.shape[0]
    nbins = NCORES * cpc
    assert nbins * P >= N, (nbins, N)
    n_pad = math.ceil(N / P) * P
    nt = n_pad // P
    cwt = cw // P
    split_jp = min(sc * cw, n_pad)
    assert split_jp <= 2**15, (sc, cw)
    assert n_pad - split_jp <= 2**15, (sc, cw, n_pad)

    deg = np.bincount(row, minlength=N).astype(np.float64)
    assert deg.min() >= 1
    r = (1.0 / np.sqrt(deg)).astype(np.float32)

    bin_of, slot_of = _assign_bins(deg, nbins)

    # group edges by (destination bin, col table-half)
    eb = bin_of[row].astype(np.int64)
    ed = slot_of[row].astype(np.float32)
    ci = col // P
    ck = ci // cwt
    ntile_k = np.minimum(cwt, nt - ck * cwt)
    jp = ck * cw + (col % P) * ntile_k + (ci - ck * cwt)
    par = (jp >= split_jp).astype(np.int64)
    pidx = np.where(jp >= split_jp, jp - split_jp, jp).astype(np.int16)
    key = eb * 2 + par
    ordk = np.argsort(key, kind="stable")
    counts = np.bincount(key, minlength=nbins * 2)
    nba = math.ceil(int(counts[0::2].max()) / P)
    nbb = math.ceil(int(counts[1::2].max()) / P)
    assert nba > 0 and nbb > 0
    cap_a, cap_b = nba * P, nbb * P

    idx_a = np.full((nbins, cap_a), -1, np.int16)
    dlt_a = np.full((nbins, cap_a), -1.0, np.float32)
    idx_b = np.full((nbins, cap_b), -1, np.int16)
    dlt_b = np.full((nbins, cap_b), -1.0, np.float32)
    starts = np.concatenate([[0], np.cumsum(counts)])
    ks = key[ordk]
    pig = (np.arange(E) - starts[ks]).astype(np.int64)
    m = (ks & 1) == 0
    idx_a[ks[m] >> 1, pig[m]] = pidx[ordk][m]
    dlt_a[ks[m] >> 1, pig[m]] = ed[ordk][m]
    m = ~m
    idx_b[ks[m] >> 1, pig[m]] = pidx[ordk][m]
    dlt_b[ks[m] >> 1, pig[m]] = ed[ordk][m]
    # ucode truncates trailing -1 indices; keep >=1 real index per half so
    # the instruction never degenerates to zero descriptors
    idx_a[counts[0::2] == 0, 0] = 0
    idx_b[counts[1::2] == 0, 0] = 0

    nb = nba + nbb
    # deltas per bin: [P, nb] with delta[p, b] = slot of edge b*128+p (or -1)
    d_a = dlt_a.reshape(nbins, nba, P).transpose(0, 2, 1)
    d_b = dlt_b.reshape(nbins, nbb, P).transpose(0, 2, 1)
    dall = np.concatenate([d_a, d_b], axis=2)  # [nbins, P, nb]

    # gather indices per bin: wrapped in 16 partitions, replicated 8x
    def mk_idx(idx, cap):
        t = idx.reshape(nbins, cap // 16, 16).transpose(0, 2, 1)
        return np.tile(t, (1, 8, 1))  # [nbins, 128, cap // 16]

    idx_all = np.concatenate([mk_idx(idx_a, cap_a), mk_idx(idx_b, cap_b)], axis=2)

    # per-slot output scale
    rout_bins = np.zeros((nbins, P), np.float32)
    rout_bins[bin_of, slot_of] = r

    # real (non-padded) index count per bin half; the gather's count register
    # must match the number of non-negative indices exactly
    cnts = np.zeros((nbins, 2), np.int32)
    cnts[:, 0] = np.maximum(counts[0::2], 1)
    cnts[:, 1] = np.maximum(counts[1::2], 1)

    # phase-1 arrays: xT pre-scaled by rsqrt(deg), bf16
    xT = np.zeros((FEAT, n_pad), np.float32)
    xT[:, :N] = np.ascontiguousarray(x.T) * r[None, :]
    xT = xT.astype(ml_dtypes.bfloat16)
    iota = np.tile(np.arange(P, dtype=np.float16), (P, nb))

    idxc = nb * 8  # idx columns per chunk
    core_maps = []
    for dd in range(NCORES):
        b0, b1 = dd * cpc, (dd + 1) * cpc
        core_maps.append(
            {
                "xT": xT,
                "W": np.ascontiguousarray(W.astype(ml_dtypes.bfloat16)),
                "iota": iota,
                "delta": np.ascontiguousarray(
                    dall[b0:b1].transpose(1, 0, 2).reshape(P, cpc * nb)
                ).astype(np.float16),
                "idx": np.ascontiguousarray(
                    idx_all[b0:b1].transpose(1, 0, 2).reshape(P, cpc * idxc)
                ),
                "r_out": np.ascontiguousarray(rout_bins[b0:b1].T),
                "cnt": np.tile(cnts[b0:b1].reshape(1, cpc * 2), (P, 1)),
            }
        )

    gslot = bin_of.astype(np.int64) * P + slot_of.astype(np.int64)
    return core_maps, gslot, nba, nbb, n_pad


# ----------------------------------------------------------------------------
# device kernel
# ----------------------------------------------------------------------------

def _build(n_pad, cpc, nba, nbb, cw, sc):
    nt = n_pad // P
    split_jp = min(sc * cw, n_pad)
    nb = nba + nbb
    idxc = nb * 8

    nc = bacc.Bacc(None, target_bir_lowering=False, debug=False, num_swdge_queues=4)
    xT_d = nc.declare_dram_parameter("xT", [P, n_pad], BF16, isOutput=False)
    W_d = nc.declare_dram_parameter("W", [P, EMBED], BF16, isOutput=False)
    io_d = nc.declare_dram_parameter("iota", [P, nb * P], F16, isOutput=False)
    dl_d = nc.declare_dram_parameter("delta", [P, cpc * nb], F16, isOutput=False)
    ix_d = nc.declare_dram_parameter("idx", [P, cpc * idxc], I16, isOutput=False)
    ro_d = nc.declare_dram_parameter("r_out", [P, cpc], F32, isOutput=False)
    cnt_d = nc.declare_dram_parameter("cnt", [P, cpc * 2], mybir.dt.int32, isOutput=False)
    out_d = nc.declare_dram_parameter("out", [cpc * P, EMBED], F32, isOutput=True)
    # chunk-major h2 rows (see _prepare docstring)
    h2_d = nc.dram_tensor("h2buf", [P * nt, EMBED], F16)

    starts = list(range(0, n_pad, cw))
    hi_starts = [s for s in starts if s >= split_jp]
    lo_starts = [s for s in starts if s < split_jp]

    with tile.TileContext(nc) as tc:
        with (
            tc.tile_pool(name="const", bufs=1) as constp,
            tc.tile_pool(name="ph1", bufs=4) as ph1,
            tc.tile_pool(name="ps1", bufs=4, space="PSUM") as ps1,
            tc.tile_pool(name="gA", bufs=GA_BUFS) as gAp,
            tc.tile_pool(name="gB", bufs=GB_BUFS) as gBp,
            tc.tile_pool(name="sS", bufs=3) as sSp,
            tc.tile_pool(name="ps2", bufs=4, space="PSUM") as ps2,
            tc.tile_pool(name="outp", bufs=4) as outp,
        ):
            W_sb = constp.tile([P, EMBED], BF16)
            io_sb = constp.tile([P, nb, P], F16)
            dl_sb = constp.tile([P, cpc * nb], F16)
            ix_sb = constp.tile([P, cpc * idxc], I16)
            ro_sb = constp.tile([P, cpc], F32)
            cnt_sb = constp.tile([P, cpc * 2], mybir.dt.int32)
            # consts all on the scalar HW queue: the sync queue starts
            # streaming x chunks immediately (phase 1 gates the gathers)
            nc.scalar.dma_start(out=W_sb[:], in_=W_d[:])
            nc.scalar.dma_start(out=cnt_sb[:], in_=cnt_d[:])
            nc.scalar.dma_start(out=ix_sb[:], in_=ix_d[:])
            nc.scalar.dma_start(
                out=io_sb[:], in_=io_d[:].rearrange("p (a b) -> p a b", b=P)
            )
            nc.scalar.dma_start(out=dl_sb[:], in_=dl_d[:])
            nc.scalar.dma_start(out=ro_sb[:], in_=ro_d[:])

            # zero the gather buffers once: trailing -1 indices generate no
            # DMA traffic, so those rows keep stale SBUF bytes -- they are
            # masked by S=0 but must stay finite (NaN*0 poisons PSUM).
            # gpsimd is idle until the first gather, so the memsets are free.
            for _ in range(GA_BUFS):
                z = gAp.tile([P, nba, EMBED], F16, tag="gA", name="gaz")
                nc.gpsimd.memset(z[:], 0.0)
            for _ in range(GB_BUFS):
                z = gBp.tile([P, nbb, EMBED], F16, tag="gB", name="gbz")
                nc.gpsimd.memset(z[:], 0.0)

            # ---------------- phase 1: h2 = (x*r) @ W  (bf16) ---------------
            # lo chunks first: the A-head gathers only need the lo table.
            hi_writes, lo_writes = [], []
            for start in lo_starts + hi_starts:
                w = min(cw, n_pad - start)
                ntile = w // P
                xt = ph1.tile([P, cw], BF16, tag="xt")
                nc.sync.dma_start(out=xt[:, :w], in_=xT_d[:, start : start + w])
                stage = ph1.tile([P, cw], F16, tag="stage")
                for gi, g0 in enumerate(range(0, ntile, 4)):
                    gn = min(4, ntile - g0)
                    ps = ps1.tile([P, 4, P], F32)
                    for i in range(gn):
                        t = g0 + i
                        nc.tensor.matmul(
                            ps[:, i, :],
                            xt[:, t * P : (t + 1) * P],
                            W_sb[:],
                            start=True,
                            stop=True,
                        )
                    st_view = stage[:, g0 * P : (g0 + gn) * P].rearrange(
                        "p (a b) -> p a b", b=P
                    )
                    # alternate evacuation engine so neither gates phase 1
                    if gi % 2 == 0:
                        nc.scalar.copy(st_view, ps[:, :gn, :])
                    else:
                        nc.vector.tensor_copy(st_view, ps[:, :gn, :])
                # contiguous chunk-major write (scalar HW queue; x reads on sync)
                wi = nc.scalar.dma_start(
                    out=h2_d[start : start + w, :].rearrange(
                        "(p l) f -> p (l f)", l=ntile
                    ),
                    in_=stage[:, :w],
                )
                (hi_writes if start >= split_jp else lo_writes).append(wi.ins)

            # ---------------- phase 2: gather + segment-sum -----------------
            # Direct-mode SWDGE gathers.  A-half gathers wait on the lo table
            # (written first), B-half on the hi table; the first A_HEAD A
            # gathers run while the hi half of phase 1 still streams.
            lo_ap = h2_d[0:split_jp, :]
            hi_ap = h2_d[split_jp : P * nt, :]
            bar_lo = nc.sync.nop(hint="h2_lo_ready")
            for wi in lo_writes:
                add_dep_helper(bar_lo.ins, wi, reason="lo gathers wait on h2 lo")
            bar_hi = nc.sync.nop(hint="h2_hi_ready")
            for wi in hi_writes:
                add_dep_helper(bar_hi.ins, wi, reason="hi gathers wait on h2 hi")

            tiles = {}

            cregs = {}

            def issue_a(c):
                ga = gAp.tile([P, nba, EMBED], F16, tag="gA", name="ga")
                rA = nc.gpsimd.alloc_register(f"cA{c}")
                rB = nc.gpsimd.alloc_register(f"cB{c}")
                nc.gpsimd.reg_load([rA, rB], cnt_sb[0:1, 2 * c : 2 * c + 2])
                cregs[c] = rB
                g1 = nc.gpsimd.dma_gather(
                    ga[:],
                    lo_ap,
                    ix_sb[:, c * idxc : c * idxc + nba * 8],
                    nba * P,
                    rA,
                    EMBED,
                    single_packet=False,
                    queue_num=(2 * c) % 4,
                )
                add_dep_helper(g1.ins, bar_lo.ins, reason="h2 lo ready")
                return ga

            def issue_b(c):
                gb = gBp.tile([P, nbb, EMBED], F16, tag="gB", name="gb")
                rB = cregs.pop(c)
                g2 = nc.gpsimd.dma_gather(
                    gb[:],
                    hi_ap,
                    ix_sb[:, c * idxc + nba * 8 : (c + 1) * idxc],
                    nbb * P,
                    rB,
                    EMBED,
                    single_packet=False,
                    queue_num=(2 * c + 1) % 4,
                )
                add_dep_helper(g2.ins, bar_hi.ins, reason="h2 hi ready")
                return gb

            def consume(j):
                ga, gb = tiles.pop(j)
                S = sSp.tile([P, nb, P], F16, tag="S", name="S")
                dsl = (
                    dl_sb[:, j * nb : (j + 1) * nb]
                    .unsqueeze(2)
                    .to_broadcast([P, nb, P])
                )
                nc.vector.tensor_tensor(
                    out=S[:], in0=dsl, in1=io_sb[:], op=mybir.AluOpType.is_equal
                )
                ps = ps2.tile([P, EMBED], F32, tag="ps2t", name="ps2t")
                for b in range(nb):
                    rhs = ga[:, b, :] if b < nba else gb[:, b - nba, :]
                    nc.tensor.matmul(
                        ps[:],
                        S[:, b, :],
                        rhs,
                        start=(b == 0),
                        stop=(b == nb - 1),
                    )
                ot = outp.tile([P, EMBED], F32)
                nc.any.tensor_scalar(
                    out=ot[:],
                    in0=ps[:],
                    scalar1=ro_sb[:, j : j + 1],
                    scalar2=None,
                    op0=mybir.AluOpType.mult,
                )
                nc.sync.dma_start(out=out_d[j * P : (j + 1) * P, :], in_=ot[:])

            # A_HEAD A-gathers up front (they only need the lo table), then
            # alternate B/A so neither table half ever starves the engine.
            ga_tiles = {}
            for c in range(min(A_HEAD, cpc)):
                ga_tiles[c] = issue_a(c)
            for c in range(cpc):
                gb = issue_b(c)
                if c + A_HEAD < cpc:
                    ga_tiles[c + A_HEAD] = issue_a(c + A_HEAD)
                tiles[c] = (ga_tiles.pop(c), gb)
                consume(c)

    nc.compile()
    return nc


# ----------------------------------------------------------------------------
# entry point
# ----------------------------------------------------------------------------

def _run(x, W, row, col, cpc=None, cw=2048, sc=12, trace=False):
    x = np.asarray(x, np.float32)
    W = np.asarray(W, np.float32)
    row = np.asarray(row).astype(np.int64)
    col = np.asarray(col).astype(np.int64)
    N = x.shape[0]
    if cpc is None:
        cpc = math.ceil(N / (NCORES * P))
    cw = min(cw, math.ceil(N / P) * P)
    core_maps, gslot, nba, nbb, n_pad = _prepare(x, W, row, col, cpc, cw, sc)
    nc = _build(n_pad, cpc, nba, nbb, cw, sc)
    res = run_bass_kernel_spmd(
        nc, core_maps, list(range(NCORES)), trace=trace
    )
    big = np.concatenate([res.results[d]["out"] for d in range(NCORES)], axis=0)
    out = np.ascontiguousarray(big[gslot], dtype=np.float32)
    return out, res


def kernel(**inputs):
    out, _ = _run(inputs["x"], inputs["W"], inputs["row"], inputs["col"])
    return out


# revision 39
# speedup vs baseline: 1.1909x; 1.0209x over previous
"""GCN message-passing (GCNConv) on 8 Trainium2 NeuronCores via Bass/Tile.

Math (reference):
    deg[i] = |{e : row[e] == i}|
    h      = x @ W
    out[i] = sum_{e: row[e]==i} h[col[e]] / sqrt(deg[row[e]] * deg[col[e]])

The GCN norm is separable, so with r = rsqrt(deg):
    h2   = ((x * r[:, None]) @ W)                 (phase 1, on device; r folded
                                                   into x on the host)
    out  = r[:, None] * scatter_add(h2[col], row)  (phase 2)

Phase 2 is a pure gather + segment-sum (PE matmul against one-hot S matrices).

The per-edge gather runs on the GPSIMD SWDGE path whose descriptor generation
is effectively serial at ~2.6 ns/index aggregate -- it is the kernel's
critical path.  This version therefore:
  * pads index arrays with -1 and passes the real per-bin count through a
    register (the SWDGE contract: num_idxs_reg == #non-negative indices), so
    padding generates no DMA descriptors (~9% fewer);
  * runs phase 1 in bf16 (x pre-scaled by r and converted on the host), with
    evacuation split across the scalar and vector engines and DMA split
    across the sync (x reads) and scalar (h2 writes + consts) HW queues, so
    the gather tables are ready as early as possible;
  * splits the int16-limited gather table at ~50% so the A/B halves carry
    even traffic, and runs A_HEAD A-half gathers while the hi table is still
    being written;
  * pre-zeroes the gather buffers on the (otherwise idle) gpsimd engine:
    rows skipped by the -1 truncation stay finite, and S=0 masks them.

(A PREPARE_ONLY + trigger_dma variant that generates descriptors during
phase 1 was tried and reverted: prep-mode generation measured ~45% slower
per index than direct mode, and Tile's consumer waits land on DMASW lane
semaphores that prep-mode descriptors never increment, which needs post-
compile wait rewriting.  See the session notes.)
"""

import math

import numpy as np
import ml_dtypes

import concourse.bacc as bacc
import concourse.bass as bass
import concourse.mybir as mybir
import concourse.tile as tile
from concourse.bass_utils import run_bass_kernel_spmd
from concourse.tile import add_dep_helper

P = 128
NCORES = 8
FEAT = 128
EMBED = 128
F32 = mybir.dt.float32
F16 = mybir.dt.float16
BF16 = mybir.dt.bfloat16
I16 = mybir.dt.int16

A_HEAD = 5  # A-half gathers issued ahead of the B stream
GA_BUFS = A_HEAD + 3
GB_BUFS = 4


# ----------------------------------------------------------------------------
# host-side preprocessing (index plumbing + rsqrt(deg) scale factors)
# ----------------------------------------------------------------------------

def _assign_bins(deg, nbins):
    """Balanced assignment of output rows to bins (<=128 rows per bin).

    Round-based LPT: each round hands the highest-degree unassigned rows to
    the least-loaded bins that still have space.  Returns (bin_of, slot_of).
    """
    n = deg.shape[0]
    order = np.argsort(-deg, kind="stable")
    bin_of = np.empty(n, np.int32)
    slot_of = np.empty(n, np.int32)
    load = np.zeros(nbins, np.float64)
    count = np.zeros(nbins, np.int32)
    pos = 0
    while pos < n:
        avail = np.nonzero(count < P)[0]
        take = min(avail.size, n - pos)
        rows_rd = order[pos : pos + take]
        b = avail[np.argsort(load[avail], kind="stable")[:take]]
        bin_of[rows_rd] = b
        slot_of[rows_rd] = count[b]
        count[b] += 1
        load[b] += deg[rows_rd]
        pos += take
    return bin_of, slot_of


def _prepare(x, W, row, col, cpc, cw, sc):
    """Build all per-core device input arrays.  cpc = bins per core.

    The h2 table lives in DRAM in phase-1-chunk-major order: phase-1
    processes nodes in chunks of cw; within chunk k (ntile_k = cw/128 node
    tiles) node j = i*128+p (i = node tile, p = partition) sits at table row
    jp = k*cw + p*ntile_k + (i - k*cw/128) -- so each chunk's h2 write is one
    contiguous run per partition.  Gather indices are int16, so the table is
    split at jp = sc*cw; edges are routed by their half.  Index arrays are
    padded with -1: the SWDGE ucode drops trailing negative indices, so
    padding generates no DMA descriptors.
    """
    N = x.shape[0]
    E = row.shape[0]
    nbins = NCORES * cpc
    assert nbins * P >= N, (nbins, N)
    n_pad = math.ceil(N / P) * P
    nt = n_pad // P
    cwt = cw // P
    split_jp = min(sc * cw, n_pad)
    assert split_jp <= 2**15, (sc, cw)
    assert n_pad - split_jp <= 2**15, (sc, cw, n_pad)

    deg = np.bincount(row, minlength=N).astype(np.float64)
    assert deg.min() >= 1
    r = (1.0 / np.sqrt(deg)).astype(np.float32)

    bin_of, slot_of = _assign_bins(deg, nbins)

    # group edges by (destination bin, col table-half)
    eb = bin_of[row].astype(np.int64)
    ed = slot_of[row].astype(np.float32)
    ci = col // P
    ck = ci // cwt
    ntile_k = np.minimum(cwt, nt - ck * cwt)
    jp = ck * cw + (col % P) * ntile_k + (ci - ck * cwt)
    par = (jp >= split_jp).astype(np.int64)
    pidx = np.where(jp >= split_jp, jp - split_jp, jp).astype(np.int16)
    key = eb * 2 + par
    ordk = np.argsort(key, kind="stable")
    counts = np.bincount(key, minlength=nbins * 2)
    nba = math.ceil(int(counts[0::2].max()) / P)
    nbb = math.ceil(int(counts[1::2].max()) / P)
    assert nba > 0 and nbb > 0
    cap_a, cap_b = nba * P, nbb * P

    idx_a = np.full((nbins, cap_a), -1, np.int16)
    dlt_a = np.full((nbins, cap_a), -1.0, np.float32)
    idx_b = np.full((nbins, cap_b), -1, np.int16)
    dlt_b = np.full((nbins, cap_b), -1.0, np.float32)
    starts = np.concatenate([[0], np.cumsum(counts)])
    ks = key[ordk]
    pig = (np.arange(E) - starts[ks]).astype(np.int64)
    m = (ks & 1) == 0
    idx_a[ks[m] >> 1, pig[m]] = pidx[ordk][m]
    dlt_a[ks[m] >> 1, pig[m]] = ed[ordk][m]
    m = ~m
    idx_b[ks[m] >> 1, pig[m]] = pidx[ordk][m]
    dlt_b[ks[m] >> 1, pig[m]] = ed[ordk][m]
    # ucode truncates trailing -1 indices; keep >=1 real index per half so
    # the instruction never degenerates to zero descriptors
    idx_a[counts[0::2] == 0, 0] = 0
    idx_b[counts[1::2] == 0, 0] = 0

    nb = nba + nbb
    # deltas per bin: [P, nb] with delta[p, b] = slot of edge b*128+p (or -1)
    d_a = dlt_a.reshape(nbins, nba, P).transpose(0, 2, 1)
    d_b = dlt_b.reshape(nbins, nbb, P).transpose(0, 2, 1)
    dall = np.concatenate([d_a, d_b], axis=2)  # [nbins, P, nb]

    # gather indices per bin: wrapped in 16 partitions, replicated 8x
    def mk_idx(idx, cap):
        t = idx.reshape(nbins, cap // 16, 16).transpose(0, 2, 1)
        return np.tile(t, (1, 8, 1))  # [nbins, 128, cap // 16]

    idx_all = np.concatenate([mk_idx(idx_a, cap_a), mk_idx(idx_b, cap_b)], axis=2)

    # per-slot output scale
    rout_bins = np.zeros((nbins, P), np.float32)
    rout_bins[bin_of, slot_of] = r

    # real (non-padded) index count per bin half; the gather's count register
    # must match the number of non-negative indices exactly
    cnts = np.zeros((nbins, 2), np.int32)
    cnts[:, 0] = np.maximum(counts[0::2], 1)
    cnts[:, 1] = np.maximum(counts[1::2], 1)

    # phase-1 arrays: xT pre-scaled by rsqrt(deg), bf16
    xT = np.zeros((FEAT, n_pad), np.float32)
    xT[:, :N] = np.ascontiguousarray(x.T) * r[None, :]
    xT = xT.astype(ml_dtypes.bfloat16)
    iota = np.tile(np.arange(P, dtype=np.float16), (P, nb))

    idxc = nb * 8  # idx columns per chunk
    core_maps = []
    for dd in range(NCORES):
        b0, b1 = dd * cpc, (dd + 1) * cpc
        core_maps.append(
            {
                "xT": xT,
                "W": np.ascontiguousarray(W.astype(ml_dtypes.bfloat16)),
                "iota": iota,
                "delta": np.ascontiguousarray(
                    dall[b0:b1].transpose(1, 0, 2).reshape(P, cpc * nb)
                ).astype(np.float16),
                "idx": np.ascontiguousarray(
                    idx_all[b0:b1].transpose(1, 0, 2).reshape(P, cpc * idxc)
                ),
                "r_out": np.ascontiguousarray(rout_bins[b0:b1].T),
                "cnt": np.tile(cnts[b0:b1].reshape(1, cpc * 2), (P, 1)),
            }
        )

    gslot = bin_of.astype(np.int64) * P + slot_of.astype(np.int64)
    return core_maps, gslot, nba, nbb, n_pad


# ----------------------------------------------------------------------------
# device kernel
# ----------------------------------------------------------------------------

def _build(n_pad, cpc, nba, nbb, cw, sc):
    nt = n_pad // P
    split_jp = min(sc * cw, n_pad)
    nb = nba + nbb
    idxc = nb * 8

    nc = bacc.Bacc(None, target_bir_lowering=False, debug=False, num_swdge_queues=4)
    xT_d = nc.declare_dram_parameter("xT", [P, n_pad], BF16, isOutput=False)
    W_d = nc.declare_dram_parameter("W", [P, EMBED], BF16, isOutput=False)
    io_d = nc.declare_dram_parameter("iota", [P, nb * P], F16, isOutput=False)
    dl_d = nc.declare_dram_parameter("delta", [P, cpc * nb], F16, isOutput=False)
    ix_d = nc.declare_dram_parameter("idx", [P, cpc * idxc], I16, isOutput=False)
    ro_d = nc.declare_dram_parameter("r_out", [P, cpc], F32, isOutput=False)
    cnt_d = nc.declare_dram_parameter("cnt", [P, cpc * 2], mybir.dt.int32, isOutput=False)
    out_d = nc.declare_dram_parameter("out", [cpc * P, EMBED], F32, isOutput=True)
    # chunk-major h2 rows (see _prepare docstring)
    h2_d = nc.dram_tensor("h2buf", [P * nt, EMBED], F16)

    starts = list(range(0, n_pad, cw))
    hi_starts = [s for s in starts if s >= split_jp]
    lo_starts = [s for s in starts if s < split_jp]

    with tile.TileContext(nc) as tc:
        with (
            tc.tile_pool(name="const", bufs=1) as constp,
            tc.tile_pool(name="ph1", bufs=4) as ph1,
            tc.tile_pool(name="ph1s", bufs=8) as ph1s,
            tc.tile_pool(name="ps1", bufs=4, space="PSUM") as ps1,
            tc.tile_pool(name="gA", bufs=GA_BUFS) as gAp,
            tc.tile_pool(name="gB", bufs=GB_BUFS) as gBp,
            tc.tile_pool(name="sS", bufs=3) as sSp,
            tc.tile_pool(name="ps2", bufs=4, space="PSUM") as ps2,
            tc.tile_pool(name="outp", bufs=4) as outp,
        ):
            W_sb = constp.tile([P, EMBED], BF16)
            io_sb = constp.tile([P, nb, P], F16)
            dl_sb = constp.tile([P, cpc * nb], F16)
            ix_sb = constp.tile([P, cpc * idxc], I16)
            ro_sb = constp.tile([P, cpc], F32)
            cnt_sb = constp.tile([P, cpc * 2], mybir.dt.int32)
            # consts all on the scalar HW queue: the sync queue starts
            # streaming x chunks immediately (phase 1 gates the gathers)
            nc.scalar.dma_start(out=W_sb[:], in_=W_d[:])
            nc.scalar.dma_start(out=cnt_sb[:], in_=cnt_d[:])
            nc.scalar.dma_start(out=ix_sb[:], in_=ix_d[:])
            nc.scalar.dma_start(
                out=io_sb[:], in_=io_d[:].rearrange("p (a b) -> p a b", b=P)
            )
            nc.scalar.dma_start(out=dl_sb[:], in_=dl_d[:])
            nc.scalar.dma_start(out=ro_sb[:], in_=ro_d[:])

            # zero the gather buffers once: trailing -1 indices generate no
            # DMA traffic, so those rows keep stale SBUF bytes -- they are
            # masked by S=0 but must stay finite (NaN*0 poisons PSUM).
            # gpsimd is idle until the first gather, so the memsets are free.
            for _ in range(GA_BUFS):
                z = gAp.tile([P, nba, EMBED], F16, tag="gA", name="gaz")
                nc.gpsimd.memset(z[:], 0.0)
            for _ in range(GB_BUFS):
                z = gBp.tile([P, nbb, EMBED], F16, tag="gB", name="gbz")
                nc.gpsimd.memset(z[:], 0.0)

            # ---------------- phase 1: h2 = (x*r) @ W  (bf16) ---------------
            # lo chunks first: the A-head gathers only need the lo table.
            hi_writes, lo_writes = [], []
            for start in lo_starts + hi_starts:
                w = min(cw, n_pad - start)
                ntile = w // P
                xt = ph1.tile([P, cw], BF16, tag="xt")
                nc.sync.dma_start(out=xt[:, :w], in_=xT_d[:, start : start + w])
                stage = ph1s.tile([P, cw], F16, tag="stage")
                for gi, g0 in enumerate(range(0, ntile, 4)):
                    gn = min(4, ntile - g0)
                    ps = ps1.tile([P, 4, P], F32)
                    for i in range(gn):
                        t = g0 + i
                        nc.tensor.matmul(
                            ps[:, i, :],
                            xt[:, t * P : (t + 1) * P],
                            W_sb[:],
                            start=True,
                            stop=True,
                        )
                    st_view = stage[:, g0 * P : (g0 + gn) * P].rearrange(
                        "p (a b) -> p a b", b=P
                    )
                    # all evacs on vector: the scalar engine only issues DMAs,
                    # so a DMA-lane stall there can't block the PSUM pipeline
                    nc.vector.tensor_copy(st_view, ps[:, :gn, :])
                # contiguous chunk-major write (scalar HW queue; x reads on sync)
                wi = nc.scalar.dma_start(
                    out=h2_d[start : start + w, :].rearrange(
                        "(p l) f -> p (l f)", l=ntile
                    ),
                    in_=stage[:, :w],
                )
                (hi_writes if start >= split_jp else lo_writes).append(wi.ins)

            # ---------------- phase 2: gather + segment-sum -----------------
            # Direct-mode SWDGE gathers.  A-half gathers wait on the lo table
            # (written first), B-half on the hi table; the first A_HEAD A
            # gathers run while the hi half of phase 1 still streams.
            lo_ap = h2_d[0:split_jp, :]
            hi_ap = h2_d[split_jp : P * nt, :]
            bar_lo = nc.sync.nop(hint="h2_lo_ready")
            for wi in lo_writes:
                add_dep_helper(bar_lo.ins, wi, reason="lo gathers wait on h2 lo")
            bar_hi = nc.sync.nop(hint="h2_hi_ready")
            for wi in hi_writes:
                add_dep_helper(bar_hi.ins, wi, reason="hi gathers wait on h2 hi")

            tiles = {}

            cregs = {}

            def issue_a(c):
                ga = gAp.tile([P, nba, EMBED], F16, tag="gA", name="ga")
                rA = nc.gpsimd.alloc_register(f"cA{c}")
                rB = nc.gpsimd.alloc_register(f"cB{c}")
                nc.gpsimd.reg_load([rA, rB], cnt_sb[0:1, 2 * c : 2 * c + 2])
                cregs[c] = rB
                g1 = nc.gpsimd.dma_gather(
                    ga[:],
                    lo_ap,
                    ix_sb[:, c * idxc : c * idxc + nba * 8],
                    nba * P,
                    rA,
                    EMBED,
                    single_packet=False,
                    queue_num=(2 * c) % 4,
                )
                add_dep_helper(g1.ins, bar_lo.ins, reason="h2 lo ready")
                return ga

            def issue_b(c):
                gb = gBp.tile([P, nbb, EMBED], F16, tag="gB", name="gb")
                rB = cregs.pop(c)
                g2 = nc.gpsimd.dma_gather(
                    gb[:],
                    hi_ap,
                    ix_sb[:, c * idxc + nba * 8 : (c + 1) * idxc],
                    nbb * P,
                    rB,
                    EMBED,
                    single_packet=False,
                    queue_num=(2 * c + 1) % 4,
                )
                add_dep_helper(g2.ins, bar_hi.ins, reason="h2 hi ready")
                return gb

            def consume(j):
                ga, gb = tiles.pop(j)
                S = sSp.tile([P, nb, P], F16, tag="S", name="S")
                dsl = (
                    dl_sb[:, j * nb : (j + 1) * nb]
                    .unsqueeze(2)
                    .to_broadcast([P, nb, P])
                )
                nc.vector.tensor_tensor(
                    out=S[:], in0=dsl, in1=io_sb[:], op=mybir.AluOpType.is_equal
                )
                ps = ps2.tile([P, EMBED], F32, tag="ps2t", name="ps2t")
                for b in range(nb):
                    rhs = ga[:, b, :] if b < nba else gb[:, b - nba, :]
                    nc.tensor.matmul(
                        ps[:],
                        S[:, b, :],
                        rhs,
                        start=(b == 0),
                        stop=(b == nb - 1),
                    )
                ot = outp.tile([P, EMBED], F32)
                nc.any.tensor_scalar(
                    out=ot[:],
                    in0=ps[:],
                    scalar1=ro_sb[:, j : j + 1],
                    scalar2=None,
                    op0=mybir.AluOpType.mult,
                )
                nc.sync.dma_start(out=out_d[j * P : (j + 1) * P, :], in_=ot[:])

            # A_HEAD A-gathers up front (they only need the lo table), then
            # alternate B/A so neither table half ever starves the engine.
            ga_tiles = {}
            for c in range(min(A_HEAD, cpc)):
                ga_tiles[c] = issue_a(c)
            for c in range(cpc):
                gb = issue_b(c)
                if c + A_HEAD < cpc:
                    ga_tiles[c + A_HEAD] = issue_a(c + A_HEAD)
                tiles[c] = (ga_tiles.pop(c), gb)
                consume(c)

    nc.compile()
    return nc


# ----------------------------------------------------------------------------
# entry point
# ----------------------------------------------------------------------------

def _run(x, W, row, col, cpc=None, cw=2048, sc=12, trace=False):
    x = np.asarray(x, np.float32)
    W = np.asarray(W, np.float32)
    row = np.asarray(row).astype(np.int64)
    col = np.asarray(col).astype(np.int64)
    N = x.shape[0]
    if cpc is None:
        cpc = math.ceil(N / (NCORES * P))
    cw = min(cw, math.ceil(N / P) * P)
    core_maps, gslot, nba, nbb, n_pad = _prepare(x, W, row, col, cpc, cw, sc)
    nc = _build(n_pad, cpc, nba, nbb, cw, sc)
    res = run_bass_kernel_spmd(
        nc, core_maps, list(range(NCORES)), trace=trace
    )
    big = np.concatenate([res.results[d]["out"] for d in range(NCORES)], axis=0)
    out = np.ascontiguousarray(big[gslot], dtype=np.float32)
    return out, res


def kernel(**inputs):
    out, _ = _run(inputs["x"], inputs["W"], inputs["row"], inputs["col"])
    return out
